# revision 19
# baseline (speedup 1.0000x reference)
# Trainium2 Bass kernel for nn_CycleGNN (edge-partitioned GNN message passing).
#
# Fully fused single-launch design: one SPMD program over 8 cores contains the
# equery-table init, all 3 GNN layers (GRU message -> PNA aggregate -> LSTM
# edge update -> LayerNorm), the cross-core node-feature exchange (on-device
# AllGather of per-core node features between layers), per-layer target-slot
# extraction combined across cores with an AllReduce, and the JK/fc scoring
# head. The host does indexing/planning only; a single run_bass_kernel_spmd
# launch ships ~1.8MB/core of indices+weights and returns the [B,1] scores.
# (The previous 5-launch structure shipped ~500MB through the axon tunnel and
# round-tripped all edge/node state through host numpy between layers; the
# tunnel transfer + per-launch XLA/PJRT overhead dominated wall clock.)
#
# Edge-partition by dst node; nodes dealt round-robin per in-degree class so
# all 8 cores share one SPMD program. Padded node-on-partition slot layout
# makes the PNA segment sum/max/min/std full-width elementwise reductions.
# int32 indirect DMA for nfeat[src]/rel_w[etype]/equery gathers; bf16 gate
# matmuls on DMA-transposed [x|h] stacks.
#
# Cold-start cost is amortized with two persistent caches (both content-keyed,
# best-effort): the BIR JSON of the compiled program (plus a stripped-module
# sidecar so a fresh process skips the 8s Bass build and the 1s full-module
# parse), and JAX's persistent compilation cache scoped to our launch (skips
# the XLA/neuronx-cc compile of the identical program).
import sys
sys.path.insert(0, '/opt/trn_rl_repo')
import numpy as np
import ml_dtypes
from contextlib import ExitStack

import jax
from contextlib import contextmanager


@contextmanager
def _jax_cc_cache():
    import os
    # persistent XLA-executable cache scoped to our launch only: fresh
    # processes skip the neuronx/XLA compile of the (identical) program
    keys = [("jax_compilation_cache_dir", "/tmp/jax_cc_cache"),
            ("jax_persistent_cache_min_entry_size_bytes", -1),
            ("jax_persistent_cache_min_compile_time_secs", 0.0)]
    old = {}
    for k, v in keys:
        try:
            old[k] = getattr(jax.config, k)
            jax.config.update(k, v)
        except Exception:
            pass
    # NTFF tracing is unavailable on this axon client (no antenv.axon_hooks);
    # a stray BASS_TRACE=1 in the environment would crash the launch.
    old_nt = os.environ.get("BASS_NEVER_TRACE")
    os.environ["BASS_NEVER_TRACE"] = "1"
    try:
        yield
    finally:
        if old_nt is None:
            os.environ.pop("BASS_NEVER_TRACE", None)
        else:
            os.environ["BASS_NEVER_TRACE"] = old_nt
        for k, v in old.items():
            try:
                jax.config.update(k, v)
            except Exception:
                pass

import concourse.bass as bass
import concourse.tile as tile
from concourse import bacc, mybir
from concourse.bass_utils import run_bass_kernel_spmd
from concourse.masks import make_identity

f32 = mybir.dt.float32
bf16 = mybir.dt.bfloat16
i32 = mybir.dt.int32
AF = mybir.ActivationFunctionType
OP = mybir.AluOpType
AX = mybir.AxisListType
BF = ml_dtypes.bfloat16

D = 64
L = 3
NCORES = 8
EPS = 1e-5
BIG = 30000.0
CLASSES = [4, 8, 12, 16, 24, 32, 48, 64, 128]


class Plan:
    pass


def build_plan(src, dst, etype, edge_graph_id, n_nodes, nrels):
    E = src.shape[0]
    N = int(n_nodes)
    p = Plan()
    p.NR = int(nrels)
    indeg = np.bincount(dst, minlength=N)
    outdeg = np.bincount(src, minlength=N)
    p.avg_d = float(np.mean(np.log(outdeg + 1.0)))
    assert int(indeg.max()) <= CLASSES[-1]

    cls_of = np.searchsorted(CLASSES, np.maximum(indeg, 1))
    core_nodes = [[] for _ in range(NCORES)]
    gKs = []
    for ci, K in enumerate(CLASSES):
        nodes_c = np.where(cls_of == ci)[0]
        if len(nodes_c) == 0:
            continue
        percore = [nodes_c[c::NCORES] for c in range(NCORES)]
        ngroups = (max(len(x) for x in percore) + 127) // 128
        for c in range(NCORES):
            lst = list(percore[c]) + [-1] * (ngroups * 128 - len(percore[c]))
            core_nodes[c] += lst
        gKs += [K] * ngroups
    p.NL = len(core_nodes[0])
    p.G = p.NL // 128
    p.gK = gKs
    p.SK = sum(gKs)
    p.goff = np.concatenate([[0], np.cumsum(gKs)]).astype(np.int64)
    p.NTOT = NCORES * p.NL
    NL, G = p.NL, p.G

    p.gid = np.full(N, p.NTOT, dtype=np.int64)
    p.core_nodes = [np.array(cn, dtype=np.int64) for cn in core_nodes]
    for c in range(NCORES):
        cn = p.core_nodes[c]
        real = cn >= 0
        p.gid[cn[real]] = c * NL + np.where(real)[0]

    p.deginv, p.hasmsg, p.amp, p.att = [], [], [], []
    for c in range(NCORES):
        cn = p.core_nodes[c]
        dg = np.where(cn >= 0, indeg[np.maximum(cn, 0)], 0).astype(np.float64)
        ld = np.log(dg + 1.0)
        def lay(x):
            return np.ascontiguousarray(x.reshape(G, 128).T).astype(np.float32)
        p.deginv.append(lay(1.0 / np.maximum(dg, 1.0)))
        p.hasmsg.append(lay((dg > 0).astype(np.float64)))
        p.amp.append(lay(ld / p.avg_d))
        p.att.append(lay(np.where(ld > 0, p.avg_d / np.maximum(ld, EPS), 0.0)))

    # per-edge slot assignment
    order = np.argsort(dst, kind='stable')
    kfill = np.zeros(E, dtype=np.int64)
    ds = dst[order]
    runstart = np.concatenate([[0], np.where(np.diff(ds) != 0)[0] + 1])
    rl = np.diff(np.concatenate([runstart, [E]]))
    kfill[order] = np.arange(E) - np.repeat(runstart, rl)
    gidd = p.gid[dst]
    core_e = gidd // NL
    loc = gidd % NL
    part = loc % 128
    colabs = p.goff[loc // 128] + kfill
    p.ecore, p.epart, p.ecol = core_e, part, colabs
    p.frow = part * p.SK + colabs  # flat slot row in [128*SK, D] view

    p.xg_idx, p.rel_idx, p.eq_idx, p.mask = [], [], [], []
    for c in range(NCORES):
        xg = np.full((128, p.SK), p.NTOT, dtype=np.int32)
        rlx = np.full((128, p.SK), p.NR, dtype=np.int32)
        eqx = np.full((128, p.SK), 32, dtype=np.int32)
        mk = np.zeros((128, p.SK), dtype=np.float32)
        m_ = core_e == c
        xg[part[m_], colabs[m_]] = p.gid[src[m_]].astype(np.int32)
        rlx[part[m_], colabs[m_]] = etype[m_].astype(np.int32)
        eqx[part[m_], colabs[m_]] = edge_graph_id[m_].astype(np.int32)
        mk[part[m_], colabs[m_]] = 1.0
        p.xg_idx.append(xg); p.rel_idx.append(rlx); p.eq_idx.append(eqx)
        p.mask.append(mk)
    return p


def build_fused_program(p, B):
    nc = bacc.Bacc("TRN2", target_bir_lowering=False, debug=False,
                   enable_asserts=False, num_devices=NCORES)
    SK, G, NL, NTOT, NR = p.SK, p.G, p.NL, p.NTOT, p.NR
    RSLOT = 128 * SK
    B2 = 2 * B
    maxK = max(p.gK)

    din = lambda n, s, t: nc.dram_tensor(n, s, t, kind="ExternalInput").ap()

    rel_idx = din("rel_idx", [128, SK], i32)
    xg_idx = din("xg_idx", [128, SK], i32)
    eq_gidx = din("eq_gidx", [128, SK], i32)
    mask_in = din("mask", [128, SK], bf16)
    dgi_in = din("deginv", [128, G], f32)
    hm_in = din("hasmsg", [128, G], f32)
    amp_in = din("amp", [128, G], f32)
    att_in = din("att", [128, G], f32)
    w_rz3 = din("w_rz", [L, 128, 128], bf16)
    w_n3 = din("w_n", [L, 128, 128], bf16)
    w_l3 = din("w_lstm", [L, 128, 256], bf16)
    w_p3 = din("w_pna", [L, 2, 128, 192], bf16)
    rel_tab = din("rel_tab", [L * (NR + 1), D], bf16)
    tgtq_b = din("tgtq_b", [B, 2 * D], f32)
    eqp_w = din("eqp_w", [2 * D, D], f32)
    tgt_q64 = din("tgt_q64", [B2, D], f32)
    ef0_sidx = din("ef0_sidx", [B2, 1], i32)
    text_idx = din("text_idx", [B2, 1], i32)
    tmask_in = din("tmask", [B2, 1], f32)
    tnode_idx = din("tnode_idx", [B2, 1], i32)
    tnmask_in = din("tnmask", [B2, 1], f32)
    wejk = din("wejk", [3 * D, D], f32)
    wqjk = din("wqjk", [3 * D, D], f32)
    wnjk = din("wnjk", [3 * D, D], f32)
    wfc = din("wfc", [4 * D, 1], f32)
    outp = nc.dram_tensor("out", [B, 1], f32, kind="ExternalOutput").ap()

    # internal DRAM (no reuse across layers -> only RAW hazards)
    ef_bufs = [nc.dram_tensor(f"ef{l}", [RSLOT + B2, D], bf16, kind="Internal")
               for l in range(L + 1)]
    eq_bufs = [None] + [nc.dram_tensor(f"eq{l}", [RSLOT, D], bf16, kind="Internal")
                        for l in range(1, L + 1)]
    nf_res = [None] + [nc.dram_tensor(f"nfr{l}", [NL, D], f32, kind="Internal")
                       for l in range(1, L + 1)]
    nf_b16 = [nc.dram_tensor(f"nfb{l}", [NL, D], bf16, kind="Internal")
              for l in range(L - 1)]
    nf_tab = [nc.dram_tensor(f"nft{l}", [NTOT + 1, D], bf16, kind="Internal",
                             addr_space="Shared") for l in range(L - 1)]
    eq_tab = nc.dram_tensor("eq_tab", [33, D], f32, kind="Internal")
    cat_buf = nc.dram_tensor("cat_buf", [3 * B2, 3 * D], f32, kind="Internal")
    cat_shr = nc.dram_tensor("cat_shr", [3 * B2, 3 * D], f32, kind="Internal",
                             addr_space="Shared")

    def gview(dram, g):
        K = p.gK[g]
        off = int(p.goff[g])
        return (dram.ap()[0:RSLOT, :]
                .rearrange("(p s) d -> p s d", p=128)[:, off:off + K, :])

    rg = [list(range(NCORES))]

    with tile.TileContext(nc, num_cores=NCORES) as tc, ExitStack() as ctx:
        const = ctx.enter_context(tc.tile_pool(name="const", bufs=1))
        identb = const.tile([128, 128], bf16)
        make_identity(nc, identb[:])
        identf = const.tile([128, 128], f32)
        make_identity(nc, identf[:])
        epsb = const.tile([128, 1], f32)
        nc.vector.memset(epsb[:], EPS)

        def cload(shape, dt, srcap, tag):
            t = const.tile(shape, dt, tag=tag)
            nc.sync.dma_start(t[:], srcap)
            return t
        msk = cload([128, SK], bf16, mask_in[:], "c_msk")
        bgn = const.tile([128, SK], f32)
        nc.vector.tensor_scalar(out=bgn[:], in0=msk[:], scalar1=-1.0, op0=OP.add,
                                scalar2=BIG, op1=OP.mult)
        dgi = cload([128, G], f32, dgi_in[:], "c_dgi")
        hmg = cload([128, G], f32, hm_in[:], "c_hmg")
        ampt = cload([128, G], f32, amp_in[:], "c_amp")
        attt = cload([128, G], f32, att_in[:], "c_att")
        eqg = cload([128, SK], i32, eq_gidx[:], "c_eqg")
        xgi = cload([128, SK], i32, xg_idx[:], "c_xgi")
        rli0 = cload([128, SK], i32, rel_idx[:], "c_rli0")
        tix = cload([B2, 1], i32, text_idx[:], "c_tix")
        tmk = cload([B2, 1], f32, tmask_in[:], "c_tmk")
        tnix = cload([B2, 1], i32, tnode_idx[:], "c_tnix")
        tnmk = cload([B2, 1], f32, tnmask_in[:], "c_tnmk")

        # ---- init phase: zero ef0 + scatter targets, equery table, nf_tab zero rows
        with ExitStack() as ictx:
            ipool = ictx.enter_context(tc.tile_pool(name="init", bufs=1))
            ips = ictx.enter_context(tc.tile_pool(name="init_ps", bufs=1, space="PSUM"))
            z = ipool.tile([128, maxK * D], bf16)
            nc.vector.memset(z[:], 0.0)
            for g in range(G):
                K = p.gK[g]
                nc.sync.dma_start(gview(ef_bufs[0], g),
                                  z[:, 0:K * D].rearrange("p (k d) -> p k d", k=K))
            zr = ipool.tile([1, D], bf16)
            nc.vector.memset(zr[:], 0.0)
            for l in range(L - 1):
                nc.sync.dma_start(nf_tab[l].ap()[NTOT:NTOT + 1, :], zr[:])
            # equery table: tgtq_b @ eqp_w -> rows 0..B-1 of eq_tab (rest zero)
            tq = ipool.tile([B, 2 * D], f32)
            nc.sync.dma_start(tq[:], tgtq_b[:])
            tqTp = ips.tile([128, B], f32, space="PSUM")
            nc.tensor.transpose(tqTp[:], tq[:], identf[0:B, 0:B])
            tqT = ipool.tile([128, B], f32)
            nc.vector.tensor_copy(tqT[:], tqTp[:])
            wq = ipool.tile([2 * D, D], f32)
            nc.sync.dma_start(wq[:], eqp_w[:])
            oq = ips.tile([B, D], f32, space="PSUM")
            nc.tensor.matmul(oq[:], lhsT=tqT[:], rhs=wq[:], start=True, stop=True)
            ot = ipool.tile([33, D], f32)
            nc.vector.memset(ot[:], 0.0)
            nc.vector.tensor_copy(ot[0:B, :], oq[:])
            nc.sync.dma_start(eq_tab.ap()[:], ot[:])
            # scatter tgt_q rows into ef0 (non-owners write scratch rows)
            tv = ipool.tile([B2, D], f32)
            nc.sync.dma_start(tv[:], tgt_q64[:])
            tvb = ipool.tile([B2, D], bf16)
            nc.vector.tensor_copy(tvb[:], tv[:])
            six = ipool.tile([B2, 1], i32)
            nc.sync.dma_start(six[:], ef0_sidx[:])
            nc.gpsimd.indirect_dma_start(
                out=ef_bufs[0].ap(),
                out_offset=bass.IndirectOffsetOnAxis(ap=six[:], axis=0),
                in_=tvb[:], in_offset=None)

        # ---- layers
        with ExitStack() as lctx:
            gpool = lctx.enter_context(tc.tile_pool(name="grp", bufs=2))
            spool = lctx.enter_context(tc.tile_pool(name="sml", bufs=4))
            wpool = lctx.enter_context(tc.tile_pool(name="wide", bufs=3))
            wts = lctx.enter_context(tc.tile_pool(name="wts", bufs=2))
            gru_ps = lctx.enter_context(tc.tile_pool(name="gru_ps", bufs=2, space="PSUM"))
            ls_ps = lctx.enter_context(tc.tile_pool(name="ls_ps", bufs=2, space="PSUM"))
            pn_ps = lctx.enter_context(tc.tile_pool(name="pn_ps", bufs=1, space="PSUM"))

            for l in range(L):
                wrz = wts.tile([128, 128], bf16, tag="w_rz")
                nc.sync.dma_start(wrz[:], w_rz3[l])
                wn = wts.tile([128, 128], bf16, tag="w_n")
                nc.sync.dma_start(wn[:], w_n3[l])
                wl = wts.tile([128, 256], bf16, tag="w_l")
                nc.sync.dma_start(wl[:], w_l3[l])
                wp = wts.tile([128, 384], bf16, tag="w_p")
                nc.sync.dma_start(wp[:, 0:192], w_p3[l, 0])
                nc.sync.dma_start(wp[:, 192:384], w_p3[l, 1])
                if l == 0:
                    rli = rli0
                else:
                    rli = wts.tile([128, SK], i32, tag="w_rli")
                    nc.vector.tensor_scalar_add(rli[:], rli0[:], l * (NR + 1))

                for g in range(G):
                    K = p.gK[g]
                    off = int(p.goff[g])
                    KD = K * D
                    ef = gpool.tile([128, KD], bf16, tag="ef")
                    nc.sync.dma_start(ef[:].rearrange("p (k d) -> p k d", k=K),
                                      gview(ef_bufs[l], g))
                    eq = gpool.tile([128, KD], bf16, tag="eq")
                    if l == 0:
                        for k_ in range(K):
                            nc.gpsimd.indirect_dma_start(
                                out=eq[:, k_ * D:(k_ + 1) * D], out_offset=None,
                                in_=eq_tab.ap(),
                                in_offset=bass.IndirectOffsetOnAxis(
                                    ap=eqg[:, off + k_:off + k_ + 1], axis=0))
                    else:
                        nc.sync.dma_start(eq[:].rearrange("p (k d) -> p k d", k=K),
                                          gview(eq_bufs[l], g))
                    rel = gpool.tile([128, KD], bf16, tag="rel")
                    for k_ in range(K):
                        nc.gpsimd.indirect_dma_start(
                            out=rel[:, k_ * D:(k_ + 1) * D], out_offset=None,
                            in_=rel_tab[:],
                            in_offset=bass.IndirectOffsetOnAxis(
                                ap=rli[:, off + k_:off + k_ + 1], axis=0))
                    if l > 0:
                        xg = gpool.tile([128, KD], bf16, tag="xg")
                        for k_ in range(K):
                            nc.gpsimd.indirect_dma_start(
                                out=xg[:, k_ * D:(k_ + 1) * D], out_offset=None,
                                in_=nf_tab[l - 1].ap(),
                                in_offset=bass.IndirectOffsetOnAxis(
                                    ap=xgi[:, off + k_:off + k_ + 1], axis=0))
                    s_sum = gpool.tile([128, D], f32, tag="s_sum")
                    s_ssq = gpool.tile([128, D], f32, tag="s_ssq")
                    s_mx = gpool.tile([128, D], f32, tag="s_mx")
                    s_mn = gpool.tile([128, D], f32, tag="s_mn")

                    nsb = K // 4
                    for sb in range(nsb):
                        o4 = sb * 4
                        sl = slice(o4 * D, (o4 + 4) * D)
                        xh = wpool.tile([128, 512], bf16, tag="xh")
                        xhv = xh[:].rearrange("p (k t d) -> p k t d", k=4, t=2)
                        xh_x, xh_h = xhv[:, :, 0], xhv[:, :, 1]
                        eqv = eq[:, sl].rearrange("p (k d) -> p k d", k=4)
                        efv = ef[:, sl].rearrange("p (k d) -> p k d", k=4)
                        relv = rel[:, sl].rearrange("p (k d) -> p k d", k=4)
                        if l == 0:
                            nc.vector.tensor_copy(xh_x, eqv)
                        else:
                            xgv = xg[:, sl].rearrange("p (k d) -> p k d", k=4)
                            nc.vector.tensor_tensor(out=xh_x, in0=xgv, in1=eqv, op=OP.add)
                        nc.vector.tensor_tensor(out=xh_h, in0=efv, in1=relv, op=OP.mult)
                        psA = gru_ps.tile([128, 512], f32, tag="psA")
                        psB = gru_ps.tile([128, 512], f32, tag="psB")
                        for k in range(4):
                            xhT = spool.tile([128, 128], bf16, tag="xhT")
                            nc.sync.dma_start_transpose(xhT[:], xh[:, k * 128:(k + 1) * 128])
                            nc.tensor.matmul(psA[:, k * 128:(k + 1) * 128], lhsT=xhT[:],
                                             rhs=wrz[:], start=True, stop=True)
                            nc.tensor.matmul(psB[:, k * 128:(k + 1) * 128], lhsT=xhT[:],
                                             rhs=wn[:], start=True, stop=True)
                        sgA = wpool.tile([128, 512], bf16, tag="sgA")
                        nc.scalar.activation(sgA[:], psA[:], AF.Sigmoid)
                        sgAv = sgA[:].rearrange("p (k t d) -> p k t d", k=4, t=2)
                        sr, sz = sgAv[:, :, 0], sgAv[:, :, 1]
                        psBv = psB[:].rearrange("p (k t d) -> p k t d", k=4, t=2)
                        xn, hn = psBv[:, :, 0], psBv[:, :, 1]
                        rhn = wpool.tile([128, 256], f32, tag="rhn")
                        rhnv = rhn[:].rearrange("p (k d) -> p k d", k=4)
                        nc.vector.tensor_tensor(out=rhnv, in0=sr, in1=hn, op=OP.mult)
                        nin = wpool.tile([128, 256], f32, tag="nin")
                        nc.vector.tensor_tensor(out=nin[:].rearrange("p (k d) -> p k d", k=4),
                                                in0=rhnv, in1=xn, op=OP.add)
                        nn = wpool.tile([128, 256], bf16, tag="nn")
                        nc.scalar.activation(nn[:], nin[:], AF.Tanh)
                        nnv = nn[:].rearrange("p (k d) -> p k d", k=4)
                        dd = wpool.tile([128, 256], bf16, tag="dd")
                        ddv = dd[:].rearrange("p (k d) -> p k d", k=4)
                        nc.vector.tensor_tensor(out=ddv, in0=xh_h, in1=nnv, op=OP.subtract)
                        zd = wpool.tile([128, 256], bf16, tag="zd")
                        zdv = zd[:].rearrange("p (k d) -> p k d", k=4)
                        nc.vector.tensor_tensor(out=zdv, in0=sz, in1=ddv, op=OP.mult)
                        msgw = wpool.tile([128, 256], bf16, tag="msgw")
                        msgv = msgw[:].rearrange("p (k d) -> p k d", k=4)
                        nc.vector.tensor_tensor(out=msgv, in0=nnv, in1=zdv, op=OP.add)
                        mkb = msk[:, off + o4:off + o4 + 4][:, :, None].to_broadcast([128, 4, 64])
                        bgb = bgn[:, off + o4:off + o4 + 4][:, :, None].to_broadcast([128, 4, 64])
                        mxy = wpool.tile([128, 256], f32, tag="mxy")
                        mxyv = mxy[:].rearrange("p (k d) -> p k d", k=4)
                        nc.vector.tensor_tensor(out=mxyv, in0=msgv, in1=mkb, op=OP.mult)
                        mxi = wpool.tile([128, 256], f32, tag="mxi")
                        nc.vector.tensor_tensor(out=mxi[:].rearrange("p (k d) -> p k d", k=4),
                                                in0=mxyv, in1=bgb, op=OP.add)
                        mni = wpool.tile([128, 256], f32, tag="mni")
                        nc.vector.tensor_tensor(out=mni[:].rearrange("p (k d) -> p k d", k=4),
                                                in0=mxyv, in1=bgb, op=OP.subtract)
                        sqv = wpool.tile([128, 256], f32, tag="sqv")
                        nc.scalar.activation(sqv[:], mxy[:], AF.Square)

                        def kred(dst_t, src_t, op, first):
                            r = spool.tile([128, D], f32, tag="kred")
                            nc.vector.tensor_reduce(
                                out=r[:], in_=src_t[:].rearrange("p (k d) -> p d k", k=4),
                                axis=AX.X, op=op)
                            if first:
                                nc.vector.tensor_copy(dst_t[:], r[:])
                            else:
                                nc.vector.tensor_tensor(out=dst_t[:], in0=dst_t[:], in1=r[:], op=op)
                        kred(s_sum, mxy, OP.add, sb == 0)
                        kred(s_ssq, sqv, OP.add, sb == 0)
                        kred(s_mx, mxi, OP.max, sb == 0)
                        kred(s_mn, mni, OP.min, sb == 0)

                    # node phase (PNA)
                    gsl = slice(g, g + 1)
                    A = gpool.tile([128, 256], bf16, tag="A")
                    nc.vector.tensor_scalar_mul(A[:, 0:64], s_sum[:], dgi[:, gsl])
                    nc.vector.tensor_scalar_mul(A[:, 64:128], s_mx[:], hmg[:, gsl])
                    nc.vector.tensor_scalar_mul(A[:, 128:192], s_mn[:], hmg[:, gsl])
                    sqm = spool.tile([128, D], f32, tag="sqm")
                    nc.vector.tensor_scalar_mul(sqm[:], s_ssq[:], dgi[:, gsl])
                    mean_f = spool.tile([128, D], f32, tag="mean_f")
                    nc.vector.tensor_scalar_mul(mean_f[:], s_sum[:], dgi[:, gsl])
                    m2 = spool.tile([128, D], f32, tag="m2")
                    nc.vector.tensor_tensor(out=m2[:], in0=mean_f[:], in1=mean_f[:], op=OP.mult)
                    varr = spool.tile([128, D], f32, tag="varr")
                    nc.vector.tensor_tensor(out=varr[:], in0=sqm[:], in1=m2[:], op=OP.subtract)
                    nc.vector.tensor_scalar_max(varr[:], varr[:], 0.0)
                    nc.scalar.activation(A[:, 192:256], varr[:], AF.Sqrt, bias=epsb[:])
                    ccp = pn_ps.tile([128, 256], bf16, tag="ccp", space="PSUM")
                    nc.tensor.transpose(ccp[:, 0:128], A[:, 0:128], identb[:])
                    nc.tensor.transpose(ccp[:, 128:256], A[:, 128:256], identb[:])
                    c1 = spool.tile([128, 128], bf16, tag="c1")
                    c2 = spool.tile([128, 128], bf16, tag="c2")
                    nc.vector.tensor_copy(c1[:], ccp[:, 0:128])
                    nc.vector.tensor_copy(c2[:], ccp[:, 128:256])
                    pp = pn_ps.tile([128, 192], f32, tag="pp", space="PSUM")
                    for j in range(3):
                        nc.tensor.matmul(pp[:, j * 64:(j + 1) * 64], lhsT=c1[:],
                                         rhs=wp[:, j * 64:j * 64 + 64], start=True, stop=False)
                        nc.tensor.matmul(pp[:, j * 64:(j + 1) * 64], lhsT=c2[:],
                                         rhs=wp[:, 192 + j * 64:192 + j * 64 + 64],
                                         start=False, stop=True)
                    nfn = gpool.tile([128, D], f32, tag="nfn")
                    nc.vector.tensor_copy(nfn[:], pp[:, 0:64])
                    t1 = spool.tile([128, D], f32, tag="t1")
                    nc.vector.scalar_tensor_tensor(out=t1[:], in0=pp[:, 64:128],
                                                   scalar=ampt[:, gsl], op0=OP.mult,
                                                   in1=nfn[:], op1=OP.add)
                    nc.vector.scalar_tensor_tensor(out=nfn[:], in0=pp[:, 128:192],
                                                   scalar=attt[:, gsl], op0=OP.mult,
                                                   in1=t1[:], op1=OP.add)

                    def ln_cols(xt):  # LayerNorm of [128, D] f32 -> new tile
                        mr = spool.tile([128, 1], f32, tag="lnmr")
                        nc.vector.tensor_reduce(out=mr[:], in_=xt[:], axis=AX.X, op=OP.add)
                        sq = spool.tile([128, D], f32, tag="lnsq")
                        nc.scalar.activation(sq[:], xt[:], AF.Square)
                        sr_ = spool.tile([128, 1], f32, tag="lnsr")
                        nc.vector.tensor_reduce(out=sr_[:], in_=sq[:], axis=AX.X, op=OP.add)
                        mm_ = spool.tile([128, 1], f32, tag="lnmm")
                        nc.vector.tensor_scalar_mul(mm_[:], mr[:], 1.0 / D)
                        m2_ = spool.tile([128, 1], f32, tag="lnm2")
                        nc.vector.tensor_tensor(out=m2_[:], in0=mm_[:], in1=mm_[:], op=OP.mult)
                        var_ = spool.tile([128, 1], f32, tag="lnvar")
                        nc.vector.scalar_tensor_tensor(out=var_[:], in0=sr_[:], scalar=1.0 / D,
                                                       op0=OP.mult, in1=m2_[:], op1=OP.subtract)
                        sd_ = spool.tile([128, 1], f32, tag="lnsd")
                        nc.scalar.activation(sd_[:], var_[:], AF.Sqrt, bias=epsb[:])
                        rsv_ = spool.tile([128, 1], f32, tag="lnrsv")
                        nc.vector.reciprocal(rsv_[:], sd_[:])
                        negm = spool.tile([128, 1], f32, tag="lnnegm")
                        nc.vector.tensor_scalar_mul(negm[:], mm_[:], -1.0)
                        o = spool.tile([128, D], f32, tag="lnout")
                        nc.vector.tensor_scalar(out=o[:], in0=xt[:], scalar1=negm[:], op0=OP.add,
                                                scalar2=rsv_[:], op1=OP.mult)
                        return o

                    no_ = ln_cols(nfn)
                    nfr = spool.tile([128, D], f32, tag="nfr")
                    if l == 0:
                        nc.vector.tensor_copy(nfr[:], no_[:])
                    else:
                        nfl = spool.tile([128, D], f32, tag="nfl")
                        nc.sync.dma_start(nfl[:], nf_res[l].ap()[g * 128:(g + 1) * 128, :])
                        nc.vector.tensor_tensor(out=nfr[:], in0=nfl[:], in1=no_[:], op=OP.add)
                    nc.sync.dma_start(nf_res[l + 1].ap()[g * 128:(g + 1) * 128, :], nfr[:])
                    if l < L - 1:
                        nfrb = spool.tile([128, D], bf16, tag="nfrb")
                        nc.vector.tensor_copy(nfrb[:], nfr[:])
                        nc.sync.dma_start(nf_b16[l].ap()[g * 128:(g + 1) * 128, :], nfrb[:])

                    # LSTM phase
                    hhbuf = gpool.tile([128, KD], f32, tag="hhbuf")
                    cbuf = gpool.tile([128, KD], f32, tag="cbuf")
                    nfnb = gpool.tile([128, D], bf16, tag="nfnb")
                    nc.vector.tensor_copy(nfnb[:], nfn[:])
                    for hb in range(K // 2):
                        k0 = hb * 2
                        xh2 = wpool.tile([128, 256], bf16, tag="xh2")
                        x2v = xh2[:].rearrange("p (k t d) -> p k t d", k=2, t=2)
                        nfb2 = nfnb[:, None, :].to_broadcast([128, 2, 64])
                        nc.vector.tensor_copy(x2v[:, :, 0], nfb2)
                        ef2 = ef[:, k0 * D:(k0 + 2) * D].rearrange("p (k d) -> p k d", k=2)
                        nc.vector.tensor_copy(x2v[:, :, 1], ef2)
                        psL = ls_ps.tile([128, 512], f32, tag="psL")
                        for kk in range(2):
                            xhT = spool.tile([128, 128], bf16, tag="xh2T")
                            nc.sync.dma_start_transpose(xhT[:], xh2[:, kk * 128:(kk + 1) * 128])
                            nc.tensor.matmul(psL[:, kk * 256:(kk + 1) * 256], lhsT=xhT[:],
                                             rhs=wl[:], start=True, stop=True)
                        psLv = psL[:].rearrange("p (k q d) -> p k q d", k=2, q=4)
                        sg2 = wpool.tile([128, 384], bf16, tag="sg2")
                        sg2v = sg2[:].rearrange("p (k q d) -> p k q d", k=2, q=3)
                        nc.scalar.activation(sg2v, psLv[:, :, 0:3], AF.Sigmoid)
                        tg2 = wpool.tile([128, 128], bf16, tag="tg2")
                        tg2v = tg2[:].rearrange("p (k d) -> p k d", k=2)
                        nc.scalar.activation(tg2v, psLv[:, :, 3], AF.Tanh)
                        eq2 = eq[:, k0 * D:(k0 + 2) * D].rearrange("p (k d) -> p k d", k=2)
                        p1 = wpool.tile([128, 128], f32, tag="p1")
                        p1v = p1[:].rearrange("p (k d) -> p k d", k=2)
                        nc.vector.tensor_tensor(out=p1v, in0=sg2v[:, :, 1], in1=eq2, op=OP.mult)
                        t2 = wpool.tile([128, 128], f32, tag="t2")
                        t2v = t2[:].rearrange("p (k d) -> p k d", k=2)
                        nc.vector.tensor_tensor(out=t2v, in0=sg2v[:, :, 0], in1=tg2v, op=OP.mult)
                        cv = cbuf[:, k0 * D:(k0 + 2) * D].rearrange("p (k d) -> p k d", k=2)
                        nc.vector.tensor_tensor(out=cv, in0=p1v, in1=t2v, op=OP.add)
                        tc2 = wpool.tile([128, 128], bf16, tag="tc2")
                        tc2v = tc2[:].rearrange("p (k d) -> p k d", k=2)
                        nc.scalar.activation(tc2v, cv, AF.Tanh)
                        hv = hhbuf[:, k0 * D:(k0 + 2) * D].rearrange("p (k d) -> p k d", k=2)
                        nc.vector.tensor_tensor(out=hv, in0=sg2v[:, :, 2], in1=tc2v, op=OP.mult)

                    # batched LN over all K columns + residual -> dram out
                    def ln_batch(buf, resid, outdram):
                        bufv = buf[:].rearrange("p (k d) -> p k d", k=K)
                        mr = spool.tile([128, K], f32, tag="bmr")
                        nc.vector.tensor_reduce(out=mr[:], in_=bufv, axis=AX.X, op=OP.add)
                        sq = wpool.tile([128, KD], f32, tag="bsq")
                        nc.scalar.activation(sq[:], buf[:], AF.Square)
                        sr_ = spool.tile([128, K], f32, tag="bsr")
                        nc.vector.tensor_reduce(out=sr_[:], in_=sq[:].rearrange("p (k d) -> p k d", k=K),
                                                axis=AX.X, op=OP.add)
                        mm_ = spool.tile([128, K], f32, tag="bmm")
                        nc.vector.tensor_scalar_mul(mm_[:], mr[:], 1.0 / D)
                        m2_ = spool.tile([128, K], f32, tag="bm2")
                        nc.vector.tensor_tensor(out=m2_[:], in0=mm_[:], in1=mm_[:], op=OP.mult)
                        var_ = spool.tile([128, K], f32, tag="bvar")
                        nc.vector.scalar_tensor_tensor(out=var_[:], in0=sr_[:], scalar=1.0 / D,
                                                       op0=OP.mult, in1=m2_[:], op1=OP.subtract)
                        sd_ = spool.tile([128, K], f32, tag="bsd")
                        nc.scalar.activation(sd_[:], var_[:], AF.Sqrt, bias=epsb[:])
                        rsv_ = spool.tile([128, K], f32, tag="brsv")
                        nc.vector.reciprocal(rsv_[:], sd_[:])
                        t_ = wpool.tile([128, KD], f32, tag="bt")
                        tv_ = t_[:].rearrange("p (k d) -> p k d", k=K)
                        nc.vector.tensor_tensor(out=tv_, in0=bufv,
                                                in1=mm_[:, :, None].to_broadcast([128, K, 64]),
                                                op=OP.subtract)
                        o_ = wpool.tile([128, KD], f32, tag="bo")
                        ov = o_[:].rearrange("p (k d) -> p k d", k=K)
                        nc.vector.tensor_tensor(out=ov, in0=tv_,
                                                in1=rsv_[:, :, None].to_broadcast([128, K, 64]),
                                                op=OP.mult)
                        ro = wpool.tile([128, KD], bf16, tag="bro")
                        nc.vector.tensor_tensor(out=ro[:], in0=resid[:], in1=o_[:], op=OP.add)
                        nc.sync.dma_start(outdram, ro[:].rearrange("p (k d) -> p k d", k=K))
                    ln_batch(hhbuf, ef, gview(ef_bufs[l + 1], g))
                    ln_batch(cbuf, eq, gview(eq_bufs[l + 1], g))

                # cross-core node-feature exchange for next layer
                if l < L - 1:
                    nc.gpsimd.collective_compute(
                        "AllGather", mybir.AluOpType.bypass,
                        replica_groups=rg,
                        ins=[nf_b16[l].ap()],
                        outs=[nf_tab[l].ap()[0:NTOT, :]])

                # target-slot extraction for the JK head (perm order: evens, odds)
                e_g = spool.tile([B2, D], bf16, tag="x_eg")
                nc.gpsimd.indirect_dma_start(
                    out=e_g[:], out_offset=None, in_=ef_bufs[l + 1].ap(),
                    in_offset=bass.IndirectOffsetOnAxis(ap=tix[:], axis=0))
                ee = spool.tile([B2, D], f32, tag="x_ee")
                nc.vector.tensor_scalar_mul(ee[:], e_g[:], tmk[:, 0:1])
                nc.sync.dma_start(cat_buf.ap()[0:B2, l * D:(l + 1) * D], ee[:])
                q_g = spool.tile([B2, D], bf16, tag="x_qg")
                nc.gpsimd.indirect_dma_start(
                    out=q_g[:], out_offset=None, in_=eq_bufs[l + 1].ap(),
                    in_offset=bass.IndirectOffsetOnAxis(ap=tix[:], axis=0))
                qe_ = spool.tile([B2, D], f32, tag="x_qe")
                nc.vector.tensor_scalar_mul(qe_[:], q_g[:], tmk[:, 0:1])
                nc.sync.dma_start(cat_buf.ap()[B2:2 * B2, l * D:(l + 1) * D], qe_[:])
                n_g = spool.tile([B2, D], f32, tag="x_ng")
                nc.gpsimd.indirect_dma_start(
                    out=n_g[:], out_offset=None, in_=nf_res[l + 1].ap(),
                    in_offset=bass.IndirectOffsetOnAxis(ap=tnix[:], axis=0))
                ne_ = spool.tile([B2, D], f32, tag="x_ne")
                nc.vector.tensor_scalar_mul(ne_[:], n_g[:], tnmk[:, 0:1])
                nc.sync.dma_start(cat_buf.ap()[2 * B2:3 * B2, l * D:(l + 1) * D], ne_[:])

            # combine per-core target contributions
            nc.gpsimd.collective_compute(
                "AllReduce", mybir.AluOpType.add, replica_groups=rg,
                ins=[cat_buf.ap()], outs=[cat_shr.ap()])

        # ---- tail: JK projections + fc scoring
        with ExitStack() as tctx:
            tl = tctx.enter_context(tc.tile_pool(name="tail", bufs=1))
            tps = tctx.enter_context(tc.tile_pool(name="tail_ps", bufs=1, space="PSUM"))

            def jk(rows_ap, w_ap, tag):
                c = tl.tile([B2, 3 * D], f32, tag="jc")
                nc.sync.dma_start(c[:], rows_ap)
                o = tps.tile([B2, D], f32, tag="jo", space="PSUM")
                wt = tl.tile([128, D], f32, tag="jw")
                for ch, (a, b_) in enumerate([(0, 128), (128, 192)]):
                    w_ = b_ - a
                    tp = tps.tile([128, B2], f32, tag="jt", space="PSUM")
                    nc.tensor.transpose(tp[:w_, :], c[:, a:b_], identf[0:B2, 0:B2])
                    ts_ = tl.tile([128, B2], f32, tag="js")
                    nc.vector.tensor_copy(ts_[:w_, :], tp[:w_, :])
                    nc.sync.dma_start(wt[:w_, :], w_ap[a:b_, :])
                    nc.tensor.matmul(o[:], lhsT=ts_[:w_, :], rhs=wt[:w_, :],
                                     start=(ch == 0), stop=(ch == 1))
                os_ = tl.tile([B2, D], f32, tag=f"jr{tag}")
                nc.vector.tensor_copy(os_[:], o[:])
                return os_

            ejk = jk(cat_shr.ap()[0:B2, :], wejk[:], "e")
            qjk = jk(cat_shr.ap()[B2:2 * B2, :], wqjk[:], "q")
            njk = jk(cat_shr.ap()[2 * B2:3 * B2, :], wnjk[:], "n")
            right = tl.tile([B, 4 * D], f32)
            left = tl.tile([B, 4 * D], f32)
            nc.sync.dma_start(right[:, 0:D], ejk[0:B, :])
            nc.sync.dma_start(right[:, D:2 * D], qjk[0:B, :])
            nc.sync.dma_start(right[:, 2 * D:3 * D], njk[0:B, :])
            nc.sync.dma_start(right[:, 3 * D:4 * D], njk[B:B2, :])
            nc.sync.dma_start(left[:, 0:D], ejk[B:B2, :])
            nc.sync.dma_start(left[:, D:2 * D], qjk[B:B2, :])
            nc.sync.dma_start(left[:, 2 * D:3 * D], njk[B:B2, :])
            nc.sync.dma_start(left[:, 3 * D:4 * D], njk[0:B, :])
            wf = tl.tile([128, 2], f32)
            nc.sync.dma_start(wf[:, 0:1], wfc[0:128, :])
            nc.sync.dma_start(wf[:, 1:2], wfc[128:256, :])
            res = tps.tile([B, 2], f32, space="PSUM")
            for side, t in enumerate([right, left]):
                for ch in range(2):
                    tp = tps.tile([128, B], f32, tag="ftp", space="PSUM")
                    nc.tensor.transpose(tp[:], t[:, ch * 128:(ch + 1) * 128], identf[0:B, 0:B])
                    ts_ = tl.tile([128, B], f32, tag="fts")
                    nc.vector.tensor_copy(ts_[:], tp[:])
                    nc.tensor.matmul(res[:, side:side + 1], lhsT=ts_[:], rhs=wf[:, ch:ch + 1],
                                     start=(ch == 0), stop=(ch == 1))
            res_sb = tl.tile([B, 2], f32)
            nc.vector.tensor_copy(res_sb[:], res[:])
            mx = tl.tile([B, 1], f32)
            nc.vector.tensor_tensor(out=mx[:], in0=res_sb[:, 0:1], in1=res_sb[:, 1:2], op=OP.max)
            nc.sync.dma_start(outp[:], mx[:])
    nc.compile()
    return nc


_CACHE = {}
LAST_HW_NS = None
_PROGRAM_VERSION = 2


class _NcShim:
    """Stand-in for a compiled Bacc: enough surface for the axon PJRT launch
    path (run_bass_via_pjrt + bass_exec lowering), reconstructed from cached
    BIR JSON so fresh processes skip the 8s Bass build."""
    target_bir_lowering = False
    debug = False
    dbg_addr = None
    dbg_callbacks = ()

    def __init__(self, m, jb):
        self.m = m
        self._jb = jb
        self.has_collectives = True
        self.partition_id_tensor = bass.DRamTensorHandle(
            "partition_id", [1, 1], mybir.dt.uint32)

    def to_json_bytes(self):
        return self._jb


def _program_for(p, B):
    import hashlib, os, orjson
    key = repr((p.SK, p.G, p.NL, B, p.NR, tuple(p.gK), _PROGRAM_VERSION))
    h = hashlib.sha256(key.encode()).hexdigest()[:16]
    path = f"/tmp/cyclegnn_bir_{h}.json"
    meta = f"{path}.meta"
    if os.path.exists(path) and os.path.exists(meta):
        try:
            with open(path, "rb") as f:
                jb = f.read()
            with open(meta, "rb") as f:
                mb = f.read()
            # meta = same module with blocks stripped and allocations filtered
            # to External I/O only: all the launch path reads from nc.m
            return _NcShim(mybir.module_from_json_bytes(mb), jb)
        except Exception:
            pass
    nc = build_fused_program(p, B)
    try:
        jb = nc.to_json_bytes()
        d = orjson.loads(jb)
        f0 = d["functions"][0]
        f0["blocks"] = []
        f0["allocations"] = [
            a for a in f0["allocations"]
            if a.get("kind") in ("ExternalInput", "ExternalOutput")]
        mb = orjson.dumps(d)
        tmp = f"{path}.tmp{os.getpid()}"
        with open(tmp, "wb") as f:
            f.write(jb)
        os.replace(tmp, path)
        tmp = f"{meta}.tmp{os.getpid()}"
        with open(tmp, "wb") as f:
            f.write(mb)
        os.replace(tmp, meta)
    except Exception:
        pass
    return nc


_PLAN_CACHE = {}


def kernel(**inputs):
    import hashlib
    src = np.asarray(inputs["src"]).astype(np.int64)
    dst = np.asarray(inputs["dst"]).astype(np.int64)
    etype = np.asarray(inputs["etype"]).astype(np.int64)
    egid = np.asarray(inputs["edge_graph_id"]).astype(np.int64)
    tgt = np.asarray(inputs["target_edge_idx"]).astype(np.int64)
    N = int(inputs["n_nodes"])
    B = tgt.shape[0] // 2
    B2 = 2 * B
    qe = np.asarray(inputs["query_emb"], dtype=np.float32)
    NR = qe.shape[0]
    cores = list(range(NCORES))

    hh = hashlib.sha256()
    for a in (src, dst, etype, egid, tgt):
        hh.update(a.tobytes())
    hh.update(repr((N, NR)).encode())
    digest = hh.hexdigest()

    if digest in _PLAN_CACHE:
        p, statics = _PLAN_CACHE[digest]
        SK, G, NL, NTOT = p.SK, p.G, p.NL, p.NTOT
        RSLOT = 128 * SK
    else:
        p = build_plan(src, dst, etype, egid, N, NR)
        SK, G, NL, NTOT = p.SK, p.G, p.NL, p.NTOT
        RSLOT = 128 * SK
        # per-core arrays that depend only on the graph/targets, not weights
        perm = np.concatenate([np.arange(0, B2, 2), np.arange(1, B2, 2)])
        tgt_perm = tgt[perm]
        frow_t = p.frow[tgt_perm].astype(np.int32)[:, None]
        tcore = p.ecore[tgt_perm]
        tnodes = src[tgt_perm]
        tn_gid = p.gid[tnodes]
        tnode_loc = (tn_gid % NL).astype(np.int32)[:, None]
        tnode_core = tn_gid // NL
        statics = []
        for c in cores:
            ef0_sidx = np.where(p.ecore[tgt] == c, p.frow[tgt],
                                RSLOT + np.arange(B2)).astype(np.int32)[:, None]
            statics.append(dict(
                rel_idx=p.rel_idx[c],
                xg_idx=p.xg_idx[c], eq_gidx=p.eq_idx[c],
                mask=p.mask[c].astype(BF),
                deginv=p.deginv[c], hasmsg=p.hasmsg[c],
                amp=p.amp[c], att=p.att[c],
                ef0_sidx=ef0_sidx,
                text_idx=frow_t,
                tmask=(tcore == c).astype(np.float32)[:, None],
                tnode_idx=tnode_loc,
                tnmask=(tnode_core == c).astype(np.float32)[:, None],
            ))
        _PLAN_CACHE[digest] = (p, statics)

    key = (SK, G, NL, B, NR)
    if key not in _CACHE:
        _CACHE[key] = _program_for(p, B)
    nc = _CACHE[key]

    # ---- host-side prep (indexing/stacking only)
    tgt_q = qe[etype[tgt]].astype(np.float32)                 # [2B, D]
    tgtq_b = np.ascontiguousarray(tgt_q.reshape(B, 2 * D))

    def wstack(l):
        gwx = np.asarray(inputs["gru_wx"][l], np.float32)
        gwh = np.asarray(inputs["gru_wh"][l], np.float32)
        w_rz = np.concatenate([gwx[:, 0:128], gwh[:, 0:128]], 0).astype(BF)
        wn_top = np.concatenate([gwx[:, 128:192], np.zeros((D, D), np.float32)], 1)
        wn_bot = np.concatenate([np.zeros((D, D), np.float32), gwh[:, 128:192]], 1)
        w_n = np.concatenate([wn_top, wn_bot], 0).astype(BF)
        lwx = np.asarray(inputs["lstm_wx"][l], np.float32)
        lwh = np.asarray(inputs["lstm_wh"][l], np.float32)
        perm = np.concatenate([np.arange(0, 64), np.arange(64, 128),
                               np.arange(192, 256), np.arange(128, 192)])  # i,f,o,g
        w_l = np.concatenate([lwx[:, perm], lwh[:, perm]], 0).astype(BF)
        pw = np.asarray(inputs["pna_w"][l], np.float32)  # [768, 64]
        W = pw.reshape(3, 256, 64)
        c1 = np.concatenate([W[0][0:128], W[1][0:128], W[2][0:128]], 1)
        c2 = np.concatenate([W[0][128:256], W[1][128:256], W[2][128:256]], 1)
        w_pna = np.stack([c1, c2]).astype(BF)
        rel_t = np.concatenate([np.asarray(inputs["rel_w"][l], np.float32),
                                np.zeros((1, D), np.float32)], 0).astype(BF)
        return w_rz, w_n, w_l, w_pna, rel_t

    ws = [wstack(l) for l in range(L)]
    w_rz3 = np.stack([w[0] for w in ws])
    w_n3 = np.stack([w[1] for w in ws])
    w_l3 = np.stack([w[2] for w in ws])
    w_p3 = np.stack([w[3] for w in ws])
    rel_tab = np.concatenate([w[4] for w in ws], 0)           # [3*(NR+1), D]

    dyn = dict(
        w_rz=w_rz3, w_n=w_n3, w_lstm=w_l3, w_pna=w_p3, rel_tab=rel_tab,
        tgtq_b=tgtq_b, eqp_w=np.asarray(inputs["eqp_w"], np.float32),
        tgt_q64=tgt_q,
        wejk=np.asarray(inputs["ejk_w"], np.float32),
        wqjk=np.asarray(inputs["qjk_w"], np.float32),
        wnjk=np.asarray(inputs["njk_w"], np.float32),
        wfc=np.asarray(inputs["fc_w"], np.float32),
    )
    in_maps = [{**statics[c], **dyn} for c in cores]

    with _jax_cc_cache():
        rr = run_bass_kernel_spmd(nc, in_maps, cores)
    global LAST_HW_NS
    LAST_HW_NS = getattr(rr, "exec_time_ns", None)
    return rr.results[0]["out"].astype(np.float32)
